# revision 6
# baseline (speedup 1.0000x reference)
"""Bahdanau additive attention on 8 TRN2 NeuronCores -- Fourier/harmonic kernel.

Replaces the O(T*S*D) pointwise tanh (the baseline's ACT-engine wall at
~163us/core) with a separable harmonic expansion:

    tanh(z) ~= sum_{r=1..R} c_r sin(r*om0*z),   z = wq[t,d] + uh[s,d]
    sin(r*om0*(a+b)) = sin(r*om0*a)cos(r*om0*b) + cos(r*om0*a)sin(r*om0*b)

so align[t,s] = sum_r sum_d (c_r v_d sin_ra[t,d]) cos_rb[s,d] + (...) --
2R matmuls over d on the PE instead of T*S*D tanh evals.  The sin/cos
arrays live only on the small [T,D]+[S,D] sides:

  * ACT computes half-angle bases sin/cos(om0/2 * x) (args <= 2.7, inside
    the HW sin table's exact range |x|<3).
  * DVE bootstraps s~1 = sh*ch = sin(om0 x)/2 and c^1 = 2-4sh^2 =
    2cos(om0 x), then higher harmonics via the Chebyshev recurrence
    x_r = c^1 * x_{r-1} - x_{r-2} (2 fp16 tensor_tensor ops per output,
    2x DVE mode).  The s-chain runs at half scale and the c-chain at
    double scale so products s~a*c^b == sin_a*cos_b need no fixups.
  * The b-side (all four batches' uh, sum S_eff columns) is split
    column-wise between DVE and GpSimd(Pool) to run both engines.
  * v_d (and per-r c_r) fold into the a-side chain seeds / scaled copies,
    so the b-side needs no scaling at all.

Sharding, S_eff mask truncation, the additive-mask rank-1 matmuls, fused
softmax, and the output projection epilogue are inherited from the
baseline kernel; all matmul operands are fp16 (validated: end-to-end
maxrel ~2e-3 incl. fp16 recurrence rounding vs the 2e-2 gate).
"""
import numpy as np
from contextlib import ExitStack

import concourse.bass as bass
import concourse.bacc as bacc
import concourse.mybir as mybir
import concourse.tile as tile
from concourse.bass_utils import run_bass_kernel_spmd

F32 = mybir.dt.float32
F16 = mybir.dt.float16
SIN = mybir.ActivationFunctionType.Sin
EXP = mybir.ActivationFunctionType.Exp
IDENT = mybir.ActivationFunctionType.Identity
MUL = mybir.AluOpType.mult
SUB = mybir.AluOpType.subtract
ADD = mybir.AluOpType.add
F16np = np.float16

B, T, S, D, IN = 4, 512, 512, 256, 512
NC = 8
NJ = 2
TT = 128
SEG = 64

# tanh(z) ~= sum_r C[r-1] * sin(r*OM0*z), fitted on |z|<=8.85
R = 10
OM0 = 0.31111038359970267
C = [1.228446065, -0.006547976675, 0.3088999025, -0.002168842959,
     0.105935943, 0.01095791052, 0.01873589467, 0.03190948487,
     -0.01753069574, 0.02120602054]
POOL_FRAC = 0.23   # fraction of b-side columns handled by GpSimd

_BUILT = {}
LAST_RESULT = None


def _bsplit(seffs):
    """Column split of the concatenated b-side [b0|b1|b2|b3] between the
    DVE-owned and Pool-owned tiles.  Pool gets batch3 plus a tail of batch2.
    Returns (WD, WP, rhs_map) with rhs_map[b] = [(which, off, w, col0)]."""
    total = sum(seffs)
    pt = int(round(POOL_FRAC * total))
    pw2 = min(seffs[2], max(0, pt - seffs[3]))
    wd = seffs[0] + seffs[1] + (seffs[2] - pw2)
    wp = pw2 + seffs[3]
    rhs = {
        0: [("d", 0, seffs[0], 0)],
        1: [("d", seffs[0], seffs[1], 0)],
        2: [("d", seffs[0] + seffs[1], seffs[2] - pw2, 0)]
           + ([("p", 0, pw2, seffs[2] - pw2)] if pw2 else []),
        3: [("p", pw2, seffs[3], 0)],
    }
    return wd, wp, rhs


def _build(seffs):
    nc = bacc.Bacc("TRN2", target_bir_lowering=False, debug=False,
                   enable_asserts=False, num_devices=NC)
    WD, WP, RHS = _bsplit(seffs)

    xT_d = nc.dram_tensor("xT", [NJ, 4, 128, TT], F16, kind="ExternalInput")
    memsT_d = nc.dram_tensor("memsT", [4, 2, 128, S], F16, kind="ExternalInput")
    memsL_d = nc.dram_tensor("memsL", [4, 128, 4, D], F16, kind="ExternalInput")
    maskseg_d = nc.dram_tensor("maskseg", [NJ, 2, 1, S], F16, kind="ExternalInput")
    indic_d = nc.dram_tensor("indic", [2, 1, 128], F16, kind="ExternalInput")
    ones_d = nc.dram_tensor("ones1", [1, 128], F16, kind="ExternalInput")
    boutw_d = nc.dram_tensor("boutw", [1, IN], F16, kind="ExternalInput")
    WqT_d = nc.dram_tensor("WqT", [4, 128, D], F16, kind="ExternalInput")
    WcT_d = nc.dram_tensor("WcT", [2, 128, D], F16, kind="ExternalInput")
    WoCT_d = nc.dram_tensor("WoCT", [128, 2, IN], F16, kind="ExternalInput")
    WoXT_d = nc.dram_tensor("WoXT", [128, 4, IN], F16, kind="ExternalInput")
    ident_d = nc.dram_tensor("ident", [128, 128], F16, kind="ExternalInput")
    VB_d = nc.dram_tensor("VB", [128, 2 * 2 * TT], F16, kind="ExternalInput")
    VB2_d = nc.dram_tensor("VB2", [128, 2 * 2 * TT], F16, kind="ExternalInput")
    CC_d = nc.dram_tensor("CC", [128, 2], F32, kind="ExternalInput")
    CR_d = nc.dram_tensor("CR", [128, 16], F32, kind="ExternalInput")

    attn_d = nc.dram_tensor("attn_outT", [NJ, 128, 4, TT], F32, kind="ExternalOutput")
    align_d = nc.dram_tensor("align_out", [NJ, 128, S], F32, kind="ExternalOutput")

    def seff(j, k):
        return seffs[2 * j + k]

    with tile.TileContext(nc) as tc, ExitStack() as ctx:
        const = ctx.enter_context(tc.tile_pool(name="const", bufs=1))
        pin = ctx.enter_context(tc.tile_pool(name="pin", bufs=1))
        pbase = ctx.enter_context(tc.tile_pool(name="pbase", bufs=1))
        pscr = ctx.enter_context(tc.tile_pool(name="pscr", bufs=2))
        pbd = ctx.enter_context(tc.tile_pool(name="pbd", bufs=4))
        pbp = ctx.enter_context(tc.tile_pool(name="pbp", bufs=4))
        pa = ctx.enter_context(tc.tile_pool(name="pa", bufs=4))
        pw = ctx.enter_context(tc.tile_pool(name="pw", bufs=3))
        pep = ctx.enter_context(tc.tile_pool(name="pep", bufs=NJ))
        psW = ctx.enter_context(tc.tile_pool(name="psW", bufs=1, space="PSUM"))
        psU = ctx.enter_context(tc.tile_pool(name="psU", bufs=1, space="PSUM"))
        psA = ctx.enter_context(tc.tile_pool(name="psA", bufs=1, space="PSUM"))
        psT = ctx.enter_context(tc.tile_pool(name="psT", bufs=1, space="PSUM"))
        psO = ctx.enter_context(tc.tile_pool(name="psO", bufs=1, space="PSUM"))

        def load(pool, shape, dt, src, tag, engine=None):
            t = pool.tile(shape, dt, tag=tag)
            (engine or nc.sync).dma_start(t[...], src)
            return t

        # ---- input DMAs: uh(b2,b3) chain is the longest pole, load first ----
        CCt = load(const, [128, 2], F32, CC_d.ap(), "CC")
        wcTc = [load(const, [128, D], F16, WcT_d.ap()[mc], f"wcTc{mc}")
                for mc in range(2)]
        mTs = {}
        for b in [2, 3]:
            mTs[b] = [load(pin, [128, S], F16, memsT_d.ap()[b][mc], f"mT{b}c{mc}")
                      for mc in range(2)]
        wqTc = [load(const, [128, D], F16, WqT_d.ap()[ic], f"wqTc{ic}")
                for ic in range(4)]
        xTc = [[load(pin, [128, TT], F16, xT_d.ap()[j][ic], f"xT{j}c{ic}")
                for ic in range(4)] for j in range(NJ)]
        for b in [0, 1]:
            mTs[b] = [load(pin, [128, S], F16, memsT_d.ap()[b][mc], f"mT{b}c{mc}")
                      for mc in range(2)]
        VB = load(const, [128, 2 * 2 * TT], F16, VB_d.ap(), "VB")
        VB2 = load(const, [128, 2 * 2 * TT], F16, VB2_d.ap(), "VB2")
        CRt = load(const, [128, 16], F32, CR_d.ap(), "CR", nc.scalar)
        masksegs = [[load(pin, [1, S], F16, maskseg_d.ap()[j][k], f"msk{j}{k}",
                          nc.scalar)
                     for k in range(2)] for j in range(NJ)]
        indics = [load(const, [1, 128], F16, indic_d.ap()[k], f"indic{k}",
                       nc.scalar)
                  for k in range(2)]
        ones1 = load(const, [1, 128], F16, ones_d.ap(), "ones1", nc.scalar)
        boutw = load(const, [1, IN], F16, boutw_d.ap(), "boutw", nc.scalar)
        woCT = load(const, [128, 2, IN], F16, WoCT_d.ap(), "woCT", nc.scalar)
        woXT = load(const, [128, 4, IN], F16, WoXT_d.ap(), "woXT", nc.scalar)
        ident = load(const, [128, 128], F16, ident_d.ap(), "ident", nc.scalar)
        memsLs = {}
        for b in range(4):
            nch = (seffs[b] + 127) // 128
            memsLs[b] = load(pin, [128, nch, D], F16,
                             memsL_d.ap()[b][:, :nch, :], f"memsL{b}", nc.scalar)

        # ---- phase 1: wq matmuls + a-side bases ----
        AW = 2 * NJ * TT   # flat a-side width: col = h*2TT + j*TT + t
        sh_a = pbase.tile([128, AW], F16, tag="sh_a")
        ch_a = pbase.tile([128, AW], F16, tag="ch_a")
        for j in range(NJ):
            wq_ps = psW.tile([128, 2, TT], F32, tag="wqc", name=f"wq{j}")
            for h in range(2):
                for ic in range(4):
                    nc.tensor.matmul(wq_ps[:, h, :],
                                     wqTc[ic][:, h * 128:(h + 1) * 128],
                                     xTc[j][ic][...],
                                     start=(ic == 0), stop=(ic == 3))
            for h in range(2):
                a0 = h * 2 * TT + j * TT
                nc.scalar.activation(sh_a[:, a0:a0 + TT], wq_ps[:, h, :],
                                     SIN, scale=CCt[:, 0:1])
                nc.scalar.activation(ch_a[:, a0:a0 + TT], wq_ps[:, h, :],
                                     SIN, scale=CCt[:, 0:1], bias=CCt[:, 1:2])

        # a-side bootstrap (DVE): c1d_a = 2cos(om0 a) unscaled coefficient,
        # chain seeds v-scaled (recurrence is linear, v_d commutes)
        t0a = pscr.tile([128, AW], F16, tag="t0a")
        nc.vector.tensor_tensor(t0a[...], sh_a[...], sh_a[...], MUL)
        c1dd_a = pbase.tile([128, 2 * AW], F16, tag="c1dd_a")
        nc.vector.tensor_scalar(c1dd_a[:, :AW], t0a[...], -4.0, 2.0, MUL, ADD)
        nc.vector.tensor_scalar(c1dd_a[:, AW:], t0a[...], -4.0, 2.0, MUL, ADD)
        s1h_a = pscr.tile([128, AW], F16, tag="s1h_a")
        nc.vector.tensor_tensor(s1h_a[...], sh_a[...], ch_a[...], MUL)
        ag1 = pbase.tile([128, 2 * AW], F16, tag="ag1")
        nc.vector.tensor_tensor(ag1[:, :AW], s1h_a[...], VB[...], MUL)
        nc.vector.tensor_tensor(ag1[:, AW:], c1dd_a[:, :AW], VB[...], MUL)
        a_g = {1: ag1}

        # ---- phase 2: uh matmuls + b-side bases (batches 2,3 first: Pool) ----
        sh_bd = pbase.tile([128, 2 * WD], F16, tag="sh_bd")
        sh_bp = pbase.tile([128, 2 * WP], F16, tag="sh_bp")
        ch_bd = pbase.tile([128, 2 * WD], F16, tag="ch_bd")
        ch_bp = pbase.tile([128, 2 * WP], F16, tag="ch_bp")
        sh_b = {"d": sh_bd, "p": sh_bp}
        ch_b = {"d": ch_bd, "p": ch_bp}
        for b in [2, 3, 0, 1]:
            sk = seffs[b]
            uh_ps = psU.tile([128, 2, sk], F32, tag="uh",
                             padded_shape=[128, 2, S], name=f"uh{b}")
            for h in range(2):
                for mc in range(2):
                    nc.tensor.matmul(uh_ps[:, h, :],
                                     wcTc[mc][:, h * 128:(h + 1) * 128],
                                     mTs[b][mc][:, :sk],
                                     start=(mc == 0), stop=(mc == 1))
            for (which, off, w, c0) in RHS[b]:
                W = WD if which == "d" else WP
                for h in range(2):
                    nc.scalar.activation(sh_b[which][:, h * W + off:
                                                     h * W + off + w],
                                         uh_ps[:, h, c0:c0 + w], SIN,
                                         scale=CCt[:, 0:1])
                    nc.scalar.activation(ch_b[which][:, h * W + off:
                                                     h * W + off + w],
                                         uh_ps[:, h, c0:c0 + w], SIN,
                                         scale=CCt[:, 0:1], bias=CCt[:, 1:2])

        # b-side bootstrap per engine tile; generation tiles hold [s|c]
        # merged along the free axis (col = kind*2W + h*W + off) so each
        # recurrence step is 2 wide ops instead of 4
        b_g, c1dd_b = {}, {}
        for which, W, eng in ([("d", WD, nc.vector)] +
                               ([("p", WP, nc.gpsimd)] if WP else [])):
            t0 = pscr.tile([128, 2 * W], F16, tag=f"t0{which}")
            eng.tensor_tensor(t0[...], sh_b[which][...], sh_b[which][...], MUL)
            c1dd = pbase.tile([128, 4 * W], F16, tag=f"c1dd{which}")
            nc.vector.tensor_scalar(c1dd[:, :2 * W], t0[...], -4.0, 2.0, MUL, ADD)
            nc.vector.tensor_scalar(c1dd[:, 2 * W:], t0[...], -4.0, 2.0, MUL, ADD)
            g1 = pbase.tile([128, 4 * W], F16, tag=f"g1{which}")
            eng.tensor_tensor(g1[:, :2 * W], sh_b[which][...], ch_b[which][...],
                              MUL)
            nc.vector.tensor_copy(g1[:, 2 * W:], c1dd[:, :2 * W])
            c1dd_b[which] = c1dd
            b_g[which] = {1: g1}

        # ---- phase 3: harmonic chains + align matmuls ----
        al0 = psA.tile([128, S], F32, tag="al0")
        al1 = psA.tile([128, S], F32, tag="al1")
        align_pss = [al0, al1]

        def gen_b(which, r, eng):
            W = WD if which == "d" else WP
            c1dd = c1dd_b[which]
            gr = (pbd if which == "d" else pbp).tile(
                [128, 4 * W], F16, tag="bg" if which == "d" else "pg",
                name=f"bg_{which}{r}")
            eng.tensor_tensor(gr[...], c1dd[...], b_g[which][r - 1][...], MUL)
            if r > 2:
                eng.tensor_tensor(gr[...], gr[...], b_g[which][r - 2][...], SUB)
            else:
                nc.vector.tensor_scalar_add(gr[:, 2 * W:], gr[:, 2 * W:], -2.0)
            b_g[which][r] = gr
            if r >= 3:
                b_g[which].pop(r - 2)

        def gen_a(r):
            gr = pa.tile([128, 2 * AW], F16, tag="ag", name=f"ag{r}")
            nc.vector.tensor_tensor(gr[...], c1dd_a[...], a_g[r - 1][...], MUL)
            if r > 2:
                nc.vector.tensor_tensor(gr[...], gr[...], a_g[r - 2][...], SUB)
            else:
                nc.vector.tensor_tensor(gr[:, AW:], gr[:, AW:], VB2[...], SUB)
            a_g[r] = gr
            if r >= 3:
                a_g.pop(r - 2)

        for r in range(1, R + 1):
            if r >= 2:
                if WP:
                    gen_b("p", r, nc.gpsimd)
                gen_a(r)
                gen_b("d", r, nc.vector)
            wsc = pw.tile([128, 2 * AW], F16, tag="wsc", name=f"wsc{r}")
            nc.vector.tensor_scalar_mul(wsc[...], a_g[r][...], float(C[r - 1]))
            for j in range(NJ):
                for k in range(2):
                    b = 2 * j + k
                    for pi, (which, off, w, c0) in enumerate(RHS[b]):
                        W = WD if which == "d" else WP
                        for h in range(2):
                            for kind in range(2):
                                # kind 0: sin_a x cos_b; kind 1: cos_a x sin_b
                                a0 = (kind * AW + h * 2 * TT + j * TT
                                      + SEG * k)
                                lhsT = wsc[:, a0:a0 + SEG]
                                b0c = (1 - kind) * 2 * W
                                rhs = b_g[which][r][
                                    :, b0c + h * W + off: b0c + h * W + off + w]
                                first = (r == 1 and h == 0 and kind == 0
                                         and pi == 0)
                                nc.tensor.matmul(
                                    align_pss[j][SEG * k:SEG * (k + 1),
                                                 c0:c0 + w],
                                    lhsT, rhs,
                                    start=first, stop=False,
                                    tile_position=(0, SEG * k),
                                    skip_group_check=True)

        def emit_epilogue(j):
            align_ps = align_pss[j]
            for k in range(2):
                nc.tensor.matmul(align_ps[...], indics[k][...],
                                 masksegs[j][k][...],
                                 start=False, stop=(k == 1),
                                 skip_group_check=True)

            av_e = pep.tile([128, S], F32, tag="av_e")
            ssum = pep.tile([128, 1], F32, tag="ssum")
            nc.scalar.activation(av_e[...], align_ps[...], EXP,
                                 accum_out=ssum[...])

            at_ps = psO.tile([128, 4 * TT], F32, tag="at_ps")
            for oc in range(4):
                for ic in range(4):
                    nc.tensor.matmul(at_ps[:, oc * TT:(oc + 1) * TT],
                                     woXT[:, ic, oc * 128:(oc + 1) * 128],
                                     xTc[j][ic][...],
                                     start=(oc == 0 and ic == 0), stop=False)
                nc.tensor.matmul(at_ps[:, oc * TT:(oc + 1) * TT],
                                 boutw[:, oc * 128:(oc + 1) * 128],
                                 ones1[...],
                                 start=False, stop=False,
                                 skip_group_check=True)

            rcp = pep.tile([128, 1], F32, tag="rcp")
            nc.vector.reciprocal(rcp[...], ssum[...])
            av16 = pep.tile([128, S], F16, tag="av16")
            nc.vector.tensor_scalar_mul(av16[...], av_e[...], rcp[...])

            avTs = []
            for sb in range(4):
                tp = psT.tile([128, 128], F16, tag="tp")
                nc.tensor.transpose(tp[...], av16[:, sb * 128:(sb + 1) * 128],
                                    ident[...])
                avT = pep.tile([128, TT], F16, tag=f"avT{sb}")
                nc.vector.tensor_copy(avT[...], tp[...])
                avTs.append(avT)

            av = pep.tile([128, S], F32, tag="av")
            nc.vector.tensor_scalar_mul(av[...], av_e[...], rcp[...])
            nc.sync.dma_start(align_d.ap()[j], av[...])

            c_ps = psW.tile([128, 2, TT], F32, tag="wqc", name=f"c_ps{j}")
            first = True
            c_bfs = []
            for mh in range(2):
                for k in range(2):
                    b = 2 * j + k
                    nch = (seffs[b] + 127) // 128
                    for sb in range(nch):
                        nc.tensor.matmul(
                            c_ps[:, mh, k * SEG:(k + 1) * SEG],
                            memsLs[b][:, sb, mh * 128:(mh + 1) * 128],
                            avTs[sb][:, k * SEG:(k + 1) * SEG],
                            start=first, stop=False,
                            skip_group_check=True)
                        first = False
                c_bf = pep.tile([128, TT], F16, tag=f"c_bf{mh}")
                nc.vector.tensor_copy(c_bf[...], c_ps[:, mh, :])
                c_bfs.append(c_bf)

            for oc in range(4):
                for mh in range(2):
                    nc.tensor.matmul(at_ps[:, oc * TT:(oc + 1) * TT],
                                     woCT[:, mh, oc * 128:(oc + 1) * 128],
                                     c_bfs[mh][...],
                                     start=False, stop=(oc == 3 and mh == 1))
                attn_sb = pep.tile([128, TT], F32, tag=f"attn_sb{oc}")
                nc.vector.tensor_copy(attn_sb[...],
                                      at_ps[:, oc * TT:(oc + 1) * TT])
                nc.sync.dma_start(attn_d.ap()[j][:, oc, :], attn_sb[...])

        emit_epilogue(0)
        emit_epilogue(1)

    nc.compile()
    return nc


def _to_chunks(a, nch):
    return np.ascontiguousarray(a.reshape(nch, 128, a.shape[-1]))


def _to_pcf(a, nch):
    return np.ascontiguousarray(a.reshape(nch, 128, a.shape[-1]).transpose(1, 0, 2))


def _prep_inputs(inputs, mems, mem_masks, Wq, Wc, bc, v, Wout, bout):
    x = np.ascontiguousarray(np.asarray(inputs, dtype=np.float32))
    mems = np.ascontiguousarray(np.asarray(mems, dtype=np.float32))
    L = np.asarray(mem_masks).astype(np.int64)
    Wq = np.asarray(Wq, dtype=np.float32)
    Wc = np.asarray(Wc, dtype=np.float32)
    bc = np.asarray(bc, dtype=np.float32)
    v = np.asarray(v, dtype=np.float32)
    Wout = np.asarray(Wout, dtype=np.float32)
    bout = np.asarray(bout, dtype=np.float32)
    assert np.all(bc == 0.0), "kernel folds bc into ACT bias cols; bc!=0 unsupported"

    seffs = tuple(int(min(max(((int(l) + 1) // 2) * 2, 2), S)) for l in L)

    WqT = _to_chunks(np.ascontiguousarray(Wq.T), 4).astype(F16np)
    WcT = _to_chunks(np.ascontiguousarray(Wc.T), 2).astype(F16np)
    WoCT = _to_pcf(np.ascontiguousarray(Wout[:, :D].T), 2).astype(F16np)
    WoXT = _to_pcf(np.ascontiguousarray(Wout[:, D:].T), 4).astype(F16np)
    ident = np.eye(128, dtype=np.float32).astype(F16np)
    indic = np.zeros((2, 1, 128), np.float32)
    indic[0, 0, :SEG] = 1.0
    indic[1, 0, SEG:] = 1.0

    VB = np.zeros((128, 2, 2 * TT), np.float32)
    for h in range(2):
        VB[:, h, :] = v[h * 128:(h + 1) * 128][:, None]
    VB = VB.reshape(128, 2 * 2 * TT)
    CC = np.zeros((128, 2), np.float32)
    CC[:, 0] = OM0 / 2
    CC[:, 1] = np.pi / 2
    CR = np.zeros((128, 16), np.float32)
    for r in range(R):
        CR[:, r] = C[r]

    shared = dict(WqT=WqT, WcT=WcT, WoCT=WoCT, WoXT=WoXT, ident=ident,
                  indic=indic.astype(F16np),
                  boutw=bout.reshape(1, IN).astype(F16np),
                  ones1=np.ones((1, 128), np.float32).astype(F16np),
                  VB=VB.astype(F16np), VB2=(2 * VB).astype(F16np),
                  CC=CC, CR=CR)

    memsT = np.zeros((4, 2, 128, S), np.float32)
    memsL = np.zeros((4, 128, 4, D), np.float32)
    for b in range(4):
        memsT[b] = _to_chunks(np.ascontiguousarray(mems[b].T), 2)
        memsL[b] = _to_pcf(mems[b], 4)

    in_maps = []
    for core in range(NC):
        r0 = core * SEG
        xT = np.zeros((NJ, 4, 128, TT), np.float32)
        maskseg = np.zeros((NJ, 2, 1, S), np.float32)
        for j in range(NJ):
            xrows = np.concatenate(
                [x[2 * j, r0:r0 + SEG, :], x[2 * j + 1, r0:r0 + SEG, :]], 0)
            xT[j] = _to_chunks(np.ascontiguousarray(xrows.T), 4)
            for k in range(2):
                b = 2 * j + k
                maskseg[j, k, 0, :] = np.where(np.arange(S) < int(L[b]),
                                               0.0, -30.0)
        m = dict(shared)
        m["xT"] = np.ascontiguousarray(xT).astype(F16np)
        m["memsT"] = np.ascontiguousarray(memsT).astype(F16np)
        m["memsL"] = np.ascontiguousarray(memsL).astype(F16np)
        m["maskseg"] = np.ascontiguousarray(maskseg).astype(F16np)
        in_maps.append(m)
    return in_maps, seffs


def kernel(**inputs):
    global LAST_RESULT
    in_maps, seffs = _prep_inputs(**inputs)
    if seffs not in _BUILT:
        _BUILT[seffs] = _build(seffs)
    res = run_bass_kernel_spmd(_BUILT[seffs], in_maps, core_ids=list(range(NC)))
    LAST_RESULT = res

    attn_h = np.zeros((B, T, IN), np.float32)
    align_v = np.zeros((B, T, S), np.float32)
    for core in range(NC):
        r0 = core * SEG
        for j in range(NJ):
            at = res.results[core]["attn_outT"][j]
            blk = np.transpose(at, (2, 1, 0)).reshape(TT, IN)
            al = res.results[core]["align_out"][j]
            for k in range(2):
                b = 2 * j + k
                attn_h[b, r0:r0 + SEG, :] = blk[k * SEG:(k + 1) * SEG]
                align_v[b, r0:r0 + SEG, :] = al[k * SEG:(k + 1) * SEG]
    return attn_h, align_v
